# revision 1
# baseline (speedup 1.0000x reference)
"""Trainium2 Bass kernel: PINN MLP forward + JVP derivatives (T, dT/dz, dT/dt, d2T/dz2).

Math (feature-major, per point):
  a1 = W1.T x + b1, h1 = tanh(a1), g = 1 - h^2
  First derivs along e_z/e_t: a1'z = W1[0,:] (per-feature const), so
    a2'z = W2.T (g1*a1'z) = W2z.T g1 with W2z = diag(W1[0]) W2 (host folded);
    same for e_t with W2t = diag(W1[1]) W2.
  Second deriv along e_z: h1'' = -2 h1 g1 (a1'z)^2, so
    a2'' = W2zz.T (h1*g1) with W2zz = -2 diag(W1[0]^2) W2 (host folded).
  Layer i>=2: t_i = g_i * a_i',  h_i'' = g_i * (a_i'' - h_i * 2(a_i'z)^2),
    a_{i+1}{',''} = W.T {t_i, h_i''}.
  Layer 3 (width 64) is packed two-deep on partitions: [dup | dup] for h3,
  [a3t | a3z] for first derivs, h3'' overwrites the upper copy of h3.
  Layer 4 (linear, M=1) is done as two [128,2] block-diag matmuls.

Sharding: pure data parallel, 262144 points -> 8 cores x 32768.
"""

import sys

import numpy as np

sys.path.insert(0, "/opt/trn_rl_repo")

import concourse.bass as bass
import concourse.bacc as bacc
import concourse.tile as tile
from concourse import mybir
from concourse.bass_utils import run_bass_kernel_spmd

N = 262144
NCORES = 8
NSH = N // NCORES  # 32768 points per core
B = 512  # points per tile
NT = NSH // B  # 64 tiles
CH = 4096  # x chunk (points) per input DMA
TPC = CH // B  # tiles per chunk

F32 = mybir.dt.float32

TRACE = False
LAST_RESULT = None


def _build():
    nc = bacc.Bacc(None, target_bir_lowering=False)

    xT = nc.declare_dram_parameter("xT", [3, NSH], F32, isOutput=False)
    W1 = nc.declare_dram_parameter("W1", [3, 128], F32, isOutput=False)
    W2 = nc.declare_dram_parameter("W2", [128, 128], F32, isOutput=False)
    W2z = nc.declare_dram_parameter("W2z", [128, 128], F32, isOutput=False)
    W2t = nc.declare_dram_parameter("W2t", [128, 128], F32, isOutput=False)
    W2zz = nc.declare_dram_parameter("W2zz", [128, 128], F32, isOutput=False)
    W3 = nc.declare_dram_parameter("W3", [128, 64], F32, isOutput=False)
    W3d = nc.declare_dram_parameter("W3d", [128, 128], F32, isOutput=False)
    Wzt = nc.declare_dram_parameter("Wzt", [128, 2], F32, isOutput=False)
    Wpp = nc.declare_dram_parameter("Wpp", [128, 2], F32, isOutput=False)
    b1 = nc.declare_dram_parameter("b1", [128, 1], F32, isOutput=False)
    b2 = nc.declare_dram_parameter("b2", [128, 1], F32, isOutput=False)
    b3s = nc.declare_dram_parameter("b3s", [128, 1], F32, isOutput=False)
    out_d = nc.declare_dram_parameter("out", [4, NSH], F32, isOutput=True)

    Tanh = mybir.ActivationFunctionType.Tanh
    Sq = mybir.ActivationFunctionType.Square
    mult = mybir.AluOpType.mult
    add = mybir.AluOpType.add
    SQ2 = float(np.sqrt(2.0))

    with tile.TileContext(nc) as tc:
        with (
            tc.tile_pool(name="consts", bufs=1) as consts,
            tc.tile_pool(name="xin", bufs=2) as xin,
            tc.tile_pool(name="work", bufs=2) as work,
            tc.tile_pool(name="psum", bufs=4, space="PSUM") as psum,
            tc.tile_pool(name="psum_o", bufs=2, space="PSUM") as psum_o,
        ):
            W1s = consts.tile([3, 128], F32)
            nc.sync.dma_start(out=W1s, in_=W1[:])
            W2s = consts.tile([128, 128], F32)
            nc.sync.dma_start(out=W2s, in_=W2[:])
            W2zs = consts.tile([128, 128], F32)
            nc.sync.dma_start(out=W2zs, in_=W2z[:])
            W2ts = consts.tile([128, 128], F32)
            nc.sync.dma_start(out=W2ts, in_=W2t[:])
            W2zzs = consts.tile([128, 128], F32)
            nc.sync.dma_start(out=W2zzs, in_=W2zz[:])
            W3s = consts.tile([128, 64], F32)
            nc.sync.dma_start(out=W3s, in_=W3[:])
            W3ds = consts.tile([128, 128], F32)
            nc.sync.dma_start(out=W3ds, in_=W3d[:])
            Wzts = consts.tile([128, 2], F32)
            nc.sync.dma_start(out=Wzts, in_=Wzt[:])
            Wpps = consts.tile([128, 2], F32)
            nc.sync.dma_start(out=Wpps, in_=Wpp[:])
            b1s = consts.tile([128, 1], F32)
            nc.sync.dma_start(out=b1s, in_=b1[:])
            b2s = consts.tile([128, 1], F32)
            nc.sync.dma_start(out=b2s, in_=b2[:])
            b3ss = consts.tile([128, 1], F32)
            nc.sync.dma_start(out=b3ss, in_=b3s[:])
            z128 = consts.tile([128, 1], F32)
            nc.vector.memset(z128, 0.0)

            xc = None
            for t in range(NT):
                ci = t % TPC
                if ci == 0:
                    c0 = (t // TPC) * CH
                    xc = xin.tile([3, CH], F32, tag="xc", name="xc")
                    nc.sync.dma_start(out=xc, in_=xT[:, c0 : c0 + CH])
                xs = xc[:, ci * B : (ci + 1) * B]

                # ---- layer 1 ----
                pa1 = psum.tile([128, B], F32, tag="ps", name="pa1")
                nc.tensor.matmul(pa1, W1s, xs)
                h1 = work.tile([128, B], F32, tag="h1", name="h1")
                nc.scalar.activation(out=h1, in_=pa1, func=Tanh, bias=b1s)
                g1 = work.tile([128, B], F32, tag="g1", name="g1")
                nc.scalar.activation(out=g1, in_=h1, func=Sq, bias=z128)
                nc.vector.tensor_scalar(
                    out=g1, in0=g1, scalar1=-1.0, scalar2=1.0, op0=mult, op1=add
                )
                u1 = work.tile([128, B], F32, tag="u1", name="u1")
                nc.vector.tensor_mul(out=u1, in0=h1, in1=g1)

                # ---- layer 2 ----
                pa2 = psum.tile([128, B], F32, tag="ps", name="pa2")
                nc.tensor.matmul(pa2, W2s, h1)
                paz = psum.tile([128, B], F32, tag="ps", name="paz")
                nc.tensor.matmul(paz, W2zs, g1)
                pat = psum.tile([128, B], F32, tag="ps", name="pat")
                nc.tensor.matmul(pat, W2ts, g1)
                papp = psum.tile([128, B], F32, tag="ps", name="papp")
                nc.tensor.matmul(papp, W2zzs, u1)

                h2 = work.tile([128, B], F32, tag="h2", name="h2")
                nc.scalar.activation(out=h2, in_=pa2, func=Tanh, bias=b2s)
                g2 = work.tile([128, B], F32, tag="g2", name="g2")
                nc.scalar.activation(out=g2, in_=h2, func=Sq, bias=z128)
                nc.vector.tensor_scalar(
                    out=g2, in0=g2, scalar1=-1.0, scalar2=1.0, op0=mult, op1=add
                )
                t2z = work.tile([128, B], F32, tag="t2z", name="t2z")
                nc.vector.tensor_mul(out=t2z, in0=g2, in1=paz)
                t2t = work.tile([128, B], F32, tag="t2t", name="t2t")
                nc.vector.tensor_mul(out=t2t, in0=g2, in1=pat)
                sq2 = work.tile([128, B], F32, tag="sq2", name="sq2")
                nc.scalar.activation(out=sq2, in_=paz, func=Sq, bias=z128, scale=SQ2)
                v2 = work.tile([128, B], F32, tag="v2", name="v2")
                nc.vector.tensor_mul(out=v2, in0=h2, in1=sq2)
                i2 = work.tile([128, B], F32, tag="i2", name="i2")
                nc.vector.tensor_sub(out=i2, in0=papp, in1=v2)
                hpp2 = work.tile([128, B], F32, tag="hpp2", name="hpp2")
                nc.vector.tensor_mul(out=hpp2, in0=g2, in1=i2)

                # ---- layer 3 (two-deep partition packing) ----
                pa3 = psum.tile([128, B], F32, tag="ps", name="pa3")
                nc.tensor.matmul(pa3, W3ds, h2)  # [a3 | a3] (dup)
                pzt = psum.tile([128, B], F32, tag="ps", name="pzt")
                nc.tensor.matmul(pzt[0:64], W3s, t2t)  # a3t on 0:64
                nc.tensor.matmul(pzt[64:128], W3s, t2z)  # a3z on 64:128
                ppp = psum.tile([128, B], F32, tag="ps", name="ppp")
                nc.tensor.matmul(ppp[64:128], W3s, hpp2)  # a3'' on 64:128

                h3 = work.tile([128, B], F32, tag="h3", name="h3")
                nc.scalar.activation(out=h3, in_=pa3, func=Tanh, bias=b3ss)
                g3 = work.tile([128, B], F32, tag="g3", name="g3")
                nc.scalar.activation(out=g3, in_=h3, func=Sq, bias=z128)
                nc.vector.tensor_scalar(
                    out=g3, in0=g3, scalar1=-1.0, scalar2=1.0, op0=mult, op1=add
                )
                t3 = work.tile([128, B], F32, tag="t3", name="t3")
                nc.vector.tensor_mul(out=t3, in0=g3, in1=pzt)  # [t3t | t3z]
                sq3 = work.tile([128, B], F32, tag="sq3", name="sq3")
                nc.scalar.activation(
                    out=sq3[64:128], in_=pzt[64:128], func=Sq,
                    bias=z128[64:128], scale=SQ2,
                )
                v3 = work.tile([128, B], F32, tag="v3", name="v3")
                nc.vector.tensor_mul(out=v3[64:128], in0=h3[64:128], in1=sq3[64:128])
                i3 = work.tile([128, B], F32, tag="i3", name="i3")
                nc.vector.tensor_sub(out=i3[64:128], in0=ppp[64:128], in1=v3[64:128])
                # h3 := [h3 | h3''] (overwrite dup half)
                nc.vector.tensor_mul(out=h3[64:128], in0=g3[64:128], in1=i3[64:128])

                # ---- layer 4 (packed block-diag) ----
                p4 = psum_o.tile([2, 2 * B], F32, tag="p4", name="p4")
                nc.tensor.matmul(p4[:, 0:B], Wzts, t3)  # rows (Tz, Tt)
                nc.tensor.matmul(p4[:, B : 2 * B], Wpps, h3)  # rows (T, Tpp)
                sb4 = work.tile([2, 2 * B], F32, tag="sb4", name="sb4")
                nc.scalar.copy(out=sb4, in_=p4)
                nc.sync.dma_start(
                    out=out_d[1:3, t * B : (t + 1) * B], in_=sb4[:, 0:B]
                )
                ofull = out_d[:]
                o03 = bass.AP(
                    tensor=ofull.tensor,
                    offset=ofull.offset + t * B,
                    ap=[[3 * NSH, 2], [1, B]],
                )
                nc.sync.dma_start(out=o03, in_=sb4[:, B : 2 * B])

    nc.finalize()
    return nc


_NC_CACHE = None


def _get_nc():
    global _NC_CACHE
    if _NC_CACHE is None:
        _NC_CACHE = _build()
    return _NC_CACHE


def kernel(**inputs):
    global LAST_RESULT
    f = np.float32
    x = np.asarray(inputs["x"], dtype=f)
    W1 = np.asarray(inputs["W1"], dtype=f)
    b1 = np.asarray(inputs["b1"], dtype=f)
    W2 = np.asarray(inputs["W2"], dtype=f)
    b2 = np.asarray(inputs["b2"], dtype=f)
    W3 = np.asarray(inputs["W3"], dtype=f)
    b3 = np.asarray(inputs["b3"], dtype=f)
    W4 = np.asarray(inputs["W4"], dtype=f)
    b4 = np.asarray(inputs["b4"], dtype=f)

    xT = np.ascontiguousarray(x.T)  # [3, N]
    w4 = W4[:, 0]
    W2z = np.ascontiguousarray(W1[0][:, None] * W2)
    W2t = np.ascontiguousarray(W1[1][:, None] * W2)
    W2zz = np.ascontiguousarray((-2.0 * W1[0] ** 2)[:, None] * W2)
    W3d = np.ascontiguousarray(np.concatenate([W3, W3], axis=1))
    Wzt = np.zeros((128, 2), f)
    Wzt[64:, 0] = w4  # row0 = W4 . t3z
    Wzt[:64, 1] = w4  # row1 = W4 . t3t
    Wpp = np.zeros((128, 2), f)
    Wpp[:64, 0] = w4  # row0 = W4 . h3
    Wpp[64:, 1] = w4  # row1 = W4 . h3''
    b3s = np.ascontiguousarray(np.concatenate([b3, b3]).reshape(128, 1))

    common = {
        "W1": W1,
        "W2": np.ascontiguousarray(W2),
        "W2z": W2z,
        "W2t": W2t,
        "W2zz": W2zz,
        "W3": np.ascontiguousarray(W3),
        "W3d": W3d,
        "Wzt": Wzt,
        "Wpp": Wpp,
        "b1": np.ascontiguousarray(b1.reshape(128, 1)),
        "b2": np.ascontiguousarray(b2.reshape(128, 1)),
        "b3s": b3s,
    }
    in_maps = [
        dict(common, xT=np.ascontiguousarray(xT[:, i * NSH : (i + 1) * NSH]))
        for i in range(NCORES)
    ]

    nc = _get_nc()
    res = run_bass_kernel_spmd(nc, in_maps, list(range(NCORES)), trace=TRACE)
    LAST_RESULT = res

    full = np.concatenate(
        [res.results[i]["out"] for i in range(NCORES)], axis=1
    )  # [4, N] rows (T, Tz, Tt, Tpp)
    out = np.ascontiguousarray(full.T).astype(f)
    out[:, 0] += b4[0]
    return out



# revision 4
# speedup vs baseline: 1.7973x; 1.7973x over previous
"""Trainium2 Bass kernel v2: PINN MLP forward + JVP derivs (T, Tz, Tt, Tzz).

Math per point (feature-major), layer l: h = tanh(a), g = 1 - h^2:
  forward:      a_{l+1} = W^T h_l
  1st deriv:    a'_{l+1} = W^T (g_l * a'_l)         (z and t directions)
  2nd deriv(z): a''_{l+1} = W^T (g_l * (a''_l - 2 h_l a'_l^2))
L1 derivative seeds fold into host-precomputed W2z/W2t/W2zz.

Perf structure vs v1:
  - All L2+ matmuls in fp16 (1 cyc/row on PE vs 4 for fp32); L1 in fp32r.
  - z-chain carries sqrt(2) (folded into W2z and the W4 output column) so
    the 2*az^2 term is a plain square -- no scale op.
  - g is never materialized: ops use (h^2 - 1) via scalar_tensor_tensor,
    with the sign folded into -W3 / -W4 weight copies.
  - (az|at) PSUM pairs are converted to fp16 SBUF by one ACT copy, making
    downstream DVE ops 2-byte (2x mode) and GPSIMD-eligible (no PSUM port).
  - i = app - v runs on the PE as an accumulating (-I) matmul.
  - Layer 3 (width 64) processes TILE PAIRS packed on partitions; layer 4
    is 4 accumulating [128,8] matmuls producing all 8 output rows.
  - Elementwise is load-balanced: ACT (tanh+converts), DVE (squares,
    products), GPSIMD (stt forms), PE (adds).

Sharding: pure data parallel, 262144 points -> 8 cores x 32768.
"""

import sys

import numpy as np

sys.path.insert(0, "/opt/trn_rl_repo")

import concourse.bass as bass
import concourse.bacc as bacc
import concourse.tile as tile
from concourse import mybir
from concourse.bass_utils import run_bass_kernel_spmd

N = 262144
NCORES = 8
NSH = N // NCORES  # 32768 points per core
B = 512  # points per tile
NT = NSH // B  # 64 tiles
NP = NT // 2  # 32 tile pairs
CH = 4096  # x chunk (points) per input DMA
TPC = CH // B  # tiles per chunk

F32 = mybir.dt.float32
F32R = mybir.dt.float32r
F16 = mybir.dt.float16

TRACE = False
LAST_RESULT = None


def _r(ap):
    return ap.bitcast(F32R)


def _build():
    nc = bacc.Bacc(None, target_bir_lowering=False)

    xT = nc.declare_dram_parameter("xT", [3, NSH], F32R, isOutput=False)
    W1 = nc.declare_dram_parameter("W1", [3, 128], F32R, isOutput=False)
    W2 = nc.declare_dram_parameter("W2", [128, 128], F16, isOutput=False)
    W2z = nc.declare_dram_parameter("W2z", [128, 128], F16, isOutput=False)
    W2t = nc.declare_dram_parameter("W2t", [128, 128], F16, isOutput=False)
    W2zz = nc.declare_dram_parameter("W2zz", [128, 128], F16, isOutput=False)
    W3 = nc.declare_dram_parameter("W3", [128, 64], F16, isOutput=False)
    W3n = nc.declare_dram_parameter("W3n", [128, 64], F16, isOutput=False)
    W4h = nc.declare_dram_parameter("W4h", [128, 8], F16, isOutput=False)
    W4z = nc.declare_dram_parameter("W4z", [128, 8], F16, isOutput=False)
    W4t = nc.declare_dram_parameter("W4t", [128, 8], F16, isOutput=False)
    W4p = nc.declare_dram_parameter("W4p", [128, 8], F16, isOutput=False)
    negI = nc.declare_dram_parameter("negI", [128, 128], F16, isOutput=False)
    b1 = nc.declare_dram_parameter("b1", [128, 1], F32, isOutput=False)
    b2 = nc.declare_dram_parameter("b2", [128, 1], F32, isOutput=False)
    b3d = nc.declare_dram_parameter("b3d", [128, 1], F32, isOutput=False)
    out_d = nc.declare_dram_parameter("out", [4, NSH], F32, isOutput=True)

    Tanh = mybir.ActivationFunctionType.Tanh
    mult = mybir.AluOpType.mult
    sub = mybir.AluOpType.subtract
    add = mybir.AluOpType.add

    with tile.TileContext(nc) as tc:
        with (
            tc.tile_pool(name="consts", bufs=1) as consts,
            tc.tile_pool(name="xin", bufs=2) as xin,
            tc.tile_pool(name="work", bufs=2) as work,
            tc.tile_pool(name="psA", bufs=2, space="PSUM") as psA,
            tc.tile_pool(name="psZT", bufs=2, space="PSUM") as psZT,
            tc.tile_pool(name="psPP", bufs=2, space="PSUM") as psPP,
        ):
            W1s = consts.tile([3, 128], F32R)
            nc.sync.dma_start(out=W1s, in_=W1[:])
            W2s = consts.tile([128, 128], F16)
            nc.sync.dma_start(out=W2s, in_=W2[:])
            W2zs = consts.tile([128, 128], F16)
            nc.sync.dma_start(out=W2zs, in_=W2z[:])
            W2ts = consts.tile([128, 128], F16)
            nc.sync.dma_start(out=W2ts, in_=W2t[:])
            W2zzs = consts.tile([128, 128], F16)
            nc.sync.dma_start(out=W2zzs, in_=W2zz[:])
            W3s = consts.tile([128, 64], F16)
            nc.sync.dma_start(out=W3s, in_=W3[:])
            W3ns = consts.tile([128, 64], F16)
            nc.sync.dma_start(out=W3ns, in_=W3n[:])
            W4hs = consts.tile([128, 8], F16)
            nc.sync.dma_start(out=W4hs, in_=W4h[:])
            W4zs = consts.tile([128, 8], F16)
            nc.sync.dma_start(out=W4zs, in_=W4z[:])
            W4ts = consts.tile([128, 8], F16)
            nc.sync.dma_start(out=W4ts, in_=W4t[:])
            W4ps = consts.tile([128, 8], F16)
            nc.sync.dma_start(out=W4ps, in_=W4p[:])
            negIs = consts.tile([128, 128], F16)
            nc.sync.dma_start(out=negIs, in_=negI[:])
            b1s = consts.tile([128, 1], F32)
            nc.sync.dma_start(out=b1s, in_=b1[:])
            b2s = consts.tile([128, 1], F32)
            nc.sync.dma_start(out=b2s, in_=b2[:])
            b3ds = consts.tile([128, 1], F32)
            nc.sync.dma_start(out=b3ds, in_=b3d[:])

            xc = None

            def half_tile(t):
                """L1+L2 for one tile; returns (h2, mtz2, mtt2, mhpp2)."""
                nonlocal xc
                ci = t % TPC
                if ci == 0:
                    c0 = (t // TPC) * CH
                    xc = xin.tile([3, CH], F32R, tag="xc", name="xc")
                    nc.sync.dma_start(out=xc, in_=xT[:, c0 : c0 + CH])
                xs = xc[:, ci * B : (ci + 1) * B]

                # ---- layer 1 ----
                pa1 = psA.tile([128, B], F32, tag="pa", name="pa1")
                nc.tensor.matmul(pa1, W1s, xs)
                h1 = work.tile([128, B], F16, tag="h1", name="h1")
                nc.scalar.activation(out=h1, in_=pa1, func=Tanh, bias=b1s)
                hh1 = work.tile([128, B], F16, tag="hh1", name="hh1")
                nc.vector.tensor_mul(out=hh1, in0=h1, in1=h1)
                g1 = work.tile([128, B], F16, tag="g1", name="g1")
                nc.vector.tensor_scalar(
                    out=g1, in0=hh1, scalar1=-1.0, scalar2=1.0, op0=mult, op1=add
                )
                u1 = work.tile([128, B], F16, tag="u1", name="u1")
                nc.vector.tensor_mul(out=u1, in0=h1, in1=g1)

                # ---- layer 2 matmuls ----
                pa2 = psA.tile([128, B], F32, tag="pa", name="pa2")
                nc.tensor.matmul(pa2, W2s, h1)
                dzt2 = psZT.tile([128, 2 * B], F32, tag="zt", name="dzt2")
                nc.tensor.matmul(dzt2[:, 0:B], W2zs, g1)  # az' = sqrt2*az
                nc.tensor.matmul(dzt2[:, B : 2 * B], W2ts, g1)  # at
                papp2 = psPP.tile([128, B], F32, tag="pp", name="papp2")
                nc.tensor.matmul(papp2, W2zzs, u1, start=True, stop=False)

                # ---- layer 2 pointwise ----
                h2 = work.tile([128, B], F16, tag="h2", name="h2")
                nc.scalar.activation(out=h2, in_=pa2, func=Tanh, bias=b2s)
                zt2 = work.tile([128, 2 * B], F16, tag="zt2", name="zt2")
                nc.scalar.copy(out=zt2, in_=dzt2)  # az'|at -> fp16 SBUF
                az2 = zt2[:, 0:B]
                at2 = zt2[:, B : 2 * B]
                mg2 = work.tile([128, B], F16, tag="mg2", name="mg2")
                nc.vector.tensor_mul(out=mg2, in0=h2, in1=h2)
                # mg2 = h2^2 - 1 = -g2
                nc.vector.tensor_scalar(
                    out=mg2, in0=mg2, scalar1=1.0, scalar2=None, op0=sub
                )
                # -t2z' = mg2 * az', -t2t = mg2 * at  (GPSIMD)
                mtz2 = work.tile([128, B], F16, tag="mtz2", name="mtz2")
                nc.gpsimd.tensor_mul(out=mtz2, in0=mg2, in1=az2)
                mtt2 = work.tile([128, B], F16, tag="mtt2", name="mtt2")
                nc.gpsimd.tensor_mul(out=mtt2, in0=mg2, in1=at2)
                sq2 = work.tile([128, B], F16, tag="sq2", name="sq2")
                nc.vector.tensor_mul(out=sq2, in0=az2, in1=az2)  # 2*az^2
                v2 = work.tile([128, B], F16, tag="v2", name="v2")
                nc.vector.tensor_mul(out=v2, in0=h2, in1=sq2)
                # papp2 <- a2'' - v2  (PE accumulate)
                nc.tensor.matmul(papp2, negIs, v2, start=False, stop=True)
                # -h2'' = mg2 * i2
                mhpp2 = work.tile([128, B], F16, tag="mhpp2", name="mhpp2")
                nc.vector.tensor_mul(out=mhpp2, in0=mg2, in1=papp2)
                return h2, mtz2, mtt2, mhpp2

            for p in range(NP):
                t0 = 2 * p
                ha, tza, tta, ppa = half_tile(t0)
                hb, tzb, ttb, ppb = half_tile(t0 + 1)

                # ---- layer 3 matmuls (pair-packed on partitions) ----
                pa3 = psA.tile([128, B], F32, tag="pa", name="pa3")
                nc.tensor.matmul(pa3[0:64], W3s, ha)
                nc.tensor.matmul(pa3[64:128], W3s, hb)
                dzt3 = psZT.tile([128, 2 * B], F32, tag="zt", name="dzt3")
                nc.tensor.matmul(dzt3[0:64, 0:B], W3ns, tza)  # a3z' tile a
                nc.tensor.matmul(dzt3[64:128, 0:B], W3ns, tzb)
                nc.tensor.matmul(dzt3[0:64, B : 2 * B], W3ns, tta)  # a3t
                nc.tensor.matmul(dzt3[64:128, B : 2 * B], W3ns, ttb)
                papp3 = psPP.tile([128, B], F32, tag="pp", name="papp3")
                nc.tensor.matmul(papp3[0:64], W3ns, ppa, start=True, stop=False)
                nc.tensor.matmul(papp3[64:128], W3ns, ppb, start=True, stop=False)

                # ---- layer 3 pointwise ----
                h3 = work.tile([128, B], F16, tag="h3", name="h3")
                nc.scalar.activation(out=h3, in_=pa3, func=Tanh, bias=b3ds)
                zt3 = work.tile([128, 2 * B], F16, tag="zt3", name="zt3")
                nc.scalar.copy(out=zt3, in_=dzt3)
                az3 = zt3[:, 0:B]
                at3 = zt3[:, B : 2 * B]
                mg3 = work.tile([128, B], F16, tag="mg3", name="mg3")
                nc.vector.tensor_mul(out=mg3, in0=h3, in1=h3)
                nc.vector.tensor_scalar(
                    out=mg3, in0=mg3, scalar1=1.0, scalar2=None, op0=sub
                )
                mtz3 = work.tile([128, B], F16, tag="mtz3", name="mtz3")
                nc.gpsimd.tensor_mul(out=mtz3, in0=mg3, in1=az3)
                mtt3 = work.tile([128, B], F16, tag="mtt3", name="mtt3")
                nc.gpsimd.tensor_mul(out=mtt3, in0=mg3, in1=at3)
                sq3 = work.tile([128, B], F16, tag="sq3", name="sq3")
                nc.vector.tensor_mul(out=sq3, in0=az3, in1=az3)
                v3 = work.tile([128, B], F16, tag="v3", name="v3")
                nc.vector.tensor_mul(out=v3, in0=h3, in1=sq3)
                nc.tensor.matmul(papp3, negIs, v3, start=False, stop=True)
                mhpp3 = work.tile([128, B], F16, tag="mhpp3", name="mhpp3")
                nc.vector.tensor_mul(out=mhpp3, in0=mg3, in1=papp3)

                # ---- layer 4: accumulate all 8 output rows ----
                p4 = psPP.tile([8, B], F32, tag="pp", name="p4")
                nc.tensor.matmul(p4, W4hs, h3, start=True, stop=False)
                nc.tensor.matmul(p4, W4zs, mtz3, start=False, stop=False)
                nc.tensor.matmul(p4, W4ts, mtt3, start=False, stop=False)
                nc.tensor.matmul(p4, W4ps, mhpp3, start=False, stop=True)
                sb4 = work.tile([8, B], F32, tag="sb4", name="sb4")
                nc.scalar.copy(out=sb4, in_=p4)
                ofull = out_d[:]
                o8 = bass.AP(
                    tensor=ofull.tensor,
                    offset=ofull.offset + t0 * B,
                    ap=[[B, 2], [NSH, 4], [1, B]],
                )
                nc.sync.dma_start(out=o8, in_=sb4)

    nc.finalize()
    return nc


_NC_CACHE = None


def _get_nc():
    global _NC_CACHE
    if _NC_CACHE is None:
        _NC_CACHE = _build()
    return _NC_CACHE


def kernel(**inputs):
    global LAST_RESULT
    f = np.float32
    f16 = np.float16
    x = np.asarray(inputs["x"], dtype=f)
    W1 = np.asarray(inputs["W1"], dtype=f)
    b1 = np.asarray(inputs["b1"], dtype=f)
    W2 = np.asarray(inputs["W2"], dtype=f)
    b2 = np.asarray(inputs["b2"], dtype=f)
    W3 = np.asarray(inputs["W3"], dtype=f)
    b3 = np.asarray(inputs["b3"], dtype=f)
    W4 = np.asarray(inputs["W4"], dtype=f)
    b4 = np.asarray(inputs["b4"], dtype=f)

    xT = np.ascontiguousarray(x.T)  # [3, N]
    w4 = W4[:, 0].astype(f)
    SQ2 = np.sqrt(2.0).astype(f)

    W4h = np.zeros((128, 8), f)
    W4h[0:64, 0] = w4
    W4h[64:128, 4] = w4
    W4z = np.zeros((128, 8), f)
    W4z[0:64, 1] = -w4 / SQ2
    W4z[64:128, 5] = -w4 / SQ2
    W4t = np.zeros((128, 8), f)
    W4t[0:64, 2] = -w4
    W4t[64:128, 6] = -w4
    W4p = np.zeros((128, 8), f)
    W4p[0:64, 3] = -w4
    W4p[64:128, 7] = -w4

    common = {
        "W1": W1,
        "W2": W2.astype(f16),
        "W2z": (SQ2 * W1[0][:, None] * W2).astype(f16),
        "W2t": (W1[1][:, None] * W2).astype(f16),
        "W2zz": (-2.0 * (W1[0] ** 2)[:, None] * W2).astype(f16),
        "W3": W3.astype(f16),
        "W3n": (-W3).astype(f16),
        "W4h": W4h.astype(f16),
        "W4z": W4z.astype(f16),
        "W4t": W4t.astype(f16),
        "W4p": W4p.astype(f16),
        "negI": (-np.eye(128)).astype(f16),
        "b1": np.ascontiguousarray(b1.reshape(128, 1)),
        "b2": np.ascontiguousarray(b2.reshape(128, 1)),
        "b3d": np.ascontiguousarray(np.concatenate([b3, b3]).reshape(128, 1)),
    }
    in_maps = [
        dict(common, xT=np.ascontiguousarray(xT[:, i * NSH : (i + 1) * NSH]))
        for i in range(NCORES)
    ]

    nc = _get_nc()
    res = run_bass_kernel_spmd(nc, in_maps, list(range(NCORES)), trace=TRACE)
    LAST_RESULT = res

    full = np.concatenate(
        [res.results[i]["out"] for i in range(NCORES)], axis=1
    )  # [4, N] rows (T, Tz, Tt, Tpp)
    out = np.ascontiguousarray(full.T).astype(f)
    out[:, 0] += b4[0]
    return out


# revision 5
# speedup vs baseline: 1.8566x; 1.0330x over previous
"""Trainium2 Bass kernel v2: PINN MLP forward + JVP derivs (T, Tz, Tt, Tzz).

Math per point (feature-major), layer l: h = tanh(a), g = 1 - h^2:
  forward:      a_{l+1} = W^T h_l
  1st deriv:    a'_{l+1} = W^T (g_l * a'_l)         (z and t directions)
  2nd deriv(z): a''_{l+1} = W^T (g_l * (a''_l - 2 h_l a'_l^2))
L1 derivative seeds fold into host-precomputed W2z/W2t/W2zz.

Perf structure vs v1:
  - All L2+ matmuls in fp16 (1 cyc/row on PE vs 4 for fp32); L1 in fp32r.
  - z-chain carries sqrt(2) (folded into W2z and the W4 output column) so
    the 2*az^2 term is a plain square -- no scale op.
  - g is never materialized: ops use (h^2 - 1) via scalar_tensor_tensor,
    with the sign folded into -W3 / -W4 weight copies.
  - (az|at) PSUM pairs are converted to fp16 SBUF by one ACT copy, making
    downstream DVE ops 2-byte (2x mode) and GPSIMD-eligible (no PSUM port).
  - i = app - v runs on the PE as an accumulating (-I) matmul.
  - Layer 3 (width 64) processes TILE PAIRS packed on partitions; layer 4
    is 4 accumulating [128,8] matmuls producing all 8 output rows.
  - Elementwise is load-balanced: ACT (tanh+converts), DVE (squares,
    products), GPSIMD (stt forms), PE (adds).

Sharding: pure data parallel, 262144 points -> 8 cores x 32768.
"""

import sys

import numpy as np

sys.path.insert(0, "/opt/trn_rl_repo")

import concourse.bass as bass
import concourse.bacc as bacc
import concourse.tile as tile
from concourse import mybir
from concourse.bass_utils import run_bass_kernel_spmd

N = 262144
NCORES = 8
NSH = N // NCORES  # 32768 points per core
B = 512  # points per tile
NT = NSH // B  # 64 tiles
NP = NT // 2  # 32 tile pairs
CH = 4096  # x chunk (points) per input DMA
TPC = CH // B  # tiles per chunk

F32 = mybir.dt.float32
F32R = mybir.dt.float32r
F16 = mybir.dt.float16

TRACE = False
LAST_RESULT = None


def _r(ap):
    return ap.bitcast(F32R)


def _build():
    nc = bacc.Bacc(None, target_bir_lowering=False)

    xT = nc.declare_dram_parameter("xT", [3, NSH], F32R, isOutput=False)
    W1 = nc.declare_dram_parameter("W1", [3, 128], F32R, isOutput=False)
    W2 = nc.declare_dram_parameter("W2", [128, 128], F16, isOutput=False)
    W2z = nc.declare_dram_parameter("W2z", [128, 128], F16, isOutput=False)
    W2t = nc.declare_dram_parameter("W2t", [128, 128], F16, isOutput=False)
    W2zz = nc.declare_dram_parameter("W2zz", [128, 128], F16, isOutput=False)
    W3 = nc.declare_dram_parameter("W3", [128, 64], F16, isOutput=False)
    W3n = nc.declare_dram_parameter("W3n", [128, 64], F16, isOutput=False)
    W4h = nc.declare_dram_parameter("W4h", [128, 8], F16, isOutput=False)
    W4z = nc.declare_dram_parameter("W4z", [128, 8], F16, isOutput=False)
    W4t = nc.declare_dram_parameter("W4t", [128, 8], F16, isOutput=False)
    W4p = nc.declare_dram_parameter("W4p", [128, 8], F16, isOutput=False)
    negI = nc.declare_dram_parameter("negI", [128, 128], F16, isOutput=False)
    b1 = nc.declare_dram_parameter("b1", [128, 1], F32, isOutput=False)
    b2 = nc.declare_dram_parameter("b2", [128, 1], F32, isOutput=False)
    b3d = nc.declare_dram_parameter("b3d", [128, 1], F32, isOutput=False)
    out_d = nc.declare_dram_parameter("out", [4, NSH], F32, isOutput=True)

    Tanh = mybir.ActivationFunctionType.Tanh
    mult = mybir.AluOpType.mult
    sub = mybir.AluOpType.subtract
    add = mybir.AluOpType.add

    with tile.TileContext(nc) as tc:
        with (
            tc.tile_pool(name="consts", bufs=1) as consts,
            tc.tile_pool(name="xin", bufs=2) as xin,
            tc.tile_pool(name="work", bufs=4) as work,
            tc.tile_pool(name="psA", bufs=2, space="PSUM") as psA,
            tc.tile_pool(name="psZT", bufs=2, space="PSUM") as psZT,
            tc.tile_pool(name="psPP", bufs=2, space="PSUM") as psPP,
        ):
            W1s = consts.tile([3, 128], F32R)
            nc.sync.dma_start(out=W1s, in_=W1[:])
            W2s = consts.tile([128, 128], F16)
            nc.sync.dma_start(out=W2s, in_=W2[:])
            W2zs = consts.tile([128, 128], F16)
            nc.sync.dma_start(out=W2zs, in_=W2z[:])
            W2ts = consts.tile([128, 128], F16)
            nc.sync.dma_start(out=W2ts, in_=W2t[:])
            W2zzs = consts.tile([128, 128], F16)
            nc.sync.dma_start(out=W2zzs, in_=W2zz[:])
            W3s = consts.tile([128, 64], F16)
            nc.sync.dma_start(out=W3s, in_=W3[:])
            W3ns = consts.tile([128, 64], F16)
            nc.sync.dma_start(out=W3ns, in_=W3n[:])
            W4hs = consts.tile([128, 8], F16)
            nc.sync.dma_start(out=W4hs, in_=W4h[:])
            W4zs = consts.tile([128, 8], F16)
            nc.sync.dma_start(out=W4zs, in_=W4z[:])
            W4ts = consts.tile([128, 8], F16)
            nc.sync.dma_start(out=W4ts, in_=W4t[:])
            W4ps = consts.tile([128, 8], F16)
            nc.sync.dma_start(out=W4ps, in_=W4p[:])
            negIs = consts.tile([128, 128], F16)
            nc.sync.dma_start(out=negIs, in_=negI[:])
            b1s = consts.tile([128, 1], F32)
            nc.sync.dma_start(out=b1s, in_=b1[:])
            b2s = consts.tile([128, 1], F32)
            nc.sync.dma_start(out=b2s, in_=b2[:])
            b3ds = consts.tile([128, 1], F32)
            nc.sync.dma_start(out=b3ds, in_=b3d[:])

            xc = None

            def half_tile(t):
                """L1+L2 for one tile; returns (h2, mtz2, mtt2, mhpp2)."""
                nonlocal xc
                ci = t % TPC
                if ci == 0:
                    c0 = (t // TPC) * CH
                    xc = xin.tile([3, CH], F32R, tag="xc", name="xc")
                    nc.sync.dma_start(out=xc, in_=xT[:, c0 : c0 + CH])
                xs = xc[:, ci * B : (ci + 1) * B]

                # ---- layer 1 ----
                pa1 = psA.tile([128, B], F32, tag="pa", name="pa1")
                nc.tensor.matmul(pa1, W1s, xs)
                h1 = work.tile([128, B], F16, tag="h1", name="h1")
                nc.scalar.activation(out=h1, in_=pa1, func=Tanh, bias=b1s)
                hh1 = work.tile([128, B], F16, tag="hh1", name="hh1")
                nc.vector.tensor_mul(out=hh1, in0=h1, in1=h1)
                g1 = work.tile([128, B], F16, tag="g1", name="g1")
                nc.vector.tensor_scalar(
                    out=g1, in0=hh1, scalar1=-1.0, scalar2=1.0, op0=mult, op1=add
                )
                u1 = work.tile([128, B], F16, tag="u1", name="u1")
                nc.vector.tensor_mul(out=u1, in0=h1, in1=g1)

                # ---- layer 2 matmuls ----
                pa2 = psA.tile([128, B], F32, tag="pa", name="pa2")
                nc.tensor.matmul(pa2, W2s, h1)
                dzt2 = psZT.tile([128, 2 * B], F32, tag="zt", name="dzt2")
                nc.tensor.matmul(dzt2[:, 0:B], W2zs, g1)  # az' = sqrt2*az
                nc.tensor.matmul(dzt2[:, B : 2 * B], W2ts, g1)  # at
                papp2 = psPP.tile([128, B], F32, tag="pp", name="papp2")
                nc.tensor.matmul(papp2, W2zzs, u1, start=True, stop=False)

                # ---- layer 2 pointwise ----
                h2 = work.tile([128, B], F16, tag="h2", name="h2")
                nc.scalar.activation(out=h2, in_=pa2, func=Tanh, bias=b2s)
                zt2 = work.tile([128, 2 * B], F16, tag="zt2", name="zt2")
                nc.scalar.copy(out=zt2, in_=dzt2)  # az'|at -> fp16 SBUF
                az2 = zt2[:, 0:B]
                at2 = zt2[:, B : 2 * B]
                mg2 = work.tile([128, B], F16, tag="mg2", name="mg2")
                nc.vector.tensor_mul(out=mg2, in0=h2, in1=h2)
                # mg2 = h2^2 - 1 = -g2
                nc.vector.tensor_scalar(
                    out=mg2, in0=mg2, scalar1=1.0, scalar2=None, op0=sub
                )
                # -t2z' = mg2 * az', -t2t = mg2 * at  (GPSIMD)
                mtz2 = work.tile([128, B], F16, tag="mtz2", name="mtz2")
                nc.gpsimd.tensor_mul(out=mtz2, in0=mg2, in1=az2)
                mtt2 = work.tile([128, B], F16, tag="mtt2", name="mtt2")
                nc.gpsimd.tensor_mul(out=mtt2, in0=mg2, in1=at2)
                sq2 = work.tile([128, B], F16, tag="sq2", name="sq2")
                nc.vector.tensor_mul(out=sq2, in0=az2, in1=az2)  # 2*az^2
                v2 = work.tile([128, B], F16, tag="v2", name="v2")
                nc.vector.tensor_mul(out=v2, in0=h2, in1=sq2)
                # papp2 <- a2'' - v2  (PE accumulate)
                nc.tensor.matmul(papp2, negIs, v2, start=False, stop=True)
                # -h2'' = mg2 * i2
                mhpp2 = work.tile([128, B], F16, tag="mhpp2", name="mhpp2")
                nc.vector.tensor_mul(out=mhpp2, in0=mg2, in1=papp2)
                return h2, mtz2, mtt2, mhpp2

            for p in range(NP):
                t0 = 2 * p
                ha, tza, tta, ppa = half_tile(t0)
                hb, tzb, ttb, ppb = half_tile(t0 + 1)

                # ---- layer 3 matmuls (pair-packed on partitions) ----
                pa3 = psA.tile([128, B], F32, tag="pa", name="pa3")
                nc.tensor.matmul(pa3[0:64], W3s, ha)
                nc.tensor.matmul(pa3[64:128], W3s, hb)
                dzt3 = psZT.tile([128, 2 * B], F32, tag="zt", name="dzt3")
                nc.tensor.matmul(dzt3[0:64, 0:B], W3ns, tza)  # a3z' tile a
                nc.tensor.matmul(dzt3[64:128, 0:B], W3ns, tzb)
                nc.tensor.matmul(dzt3[0:64, B : 2 * B], W3ns, tta)  # a3t
                nc.tensor.matmul(dzt3[64:128, B : 2 * B], W3ns, ttb)
                papp3 = psPP.tile([128, B], F32, tag="pp", name="papp3")
                nc.tensor.matmul(papp3[0:64], W3ns, ppa, start=True, stop=False)
                nc.tensor.matmul(papp3[64:128], W3ns, ppb, start=True, stop=False)

                # ---- layer 3 pointwise ----
                h3 = work.tile([128, B], F16, tag="h3", name="h3")
                nc.scalar.activation(out=h3, in_=pa3, func=Tanh, bias=b3ds)
                zt3 = work.tile([128, 2 * B], F16, tag="zt3", name="zt3")
                nc.scalar.copy(out=zt3, in_=dzt3)
                az3 = zt3[:, 0:B]
                at3 = zt3[:, B : 2 * B]
                mg3 = work.tile([128, B], F16, tag="mg3", name="mg3")
                nc.vector.tensor_mul(out=mg3, in0=h3, in1=h3)
                nc.vector.tensor_scalar(
                    out=mg3, in0=mg3, scalar1=1.0, scalar2=None, op0=sub
                )
                mtz3 = work.tile([128, B], F16, tag="mtz3", name="mtz3")
                nc.gpsimd.tensor_mul(out=mtz3, in0=mg3, in1=az3)
                mtt3 = work.tile([128, B], F16, tag="mtt3", name="mtt3")
                nc.gpsimd.tensor_mul(out=mtt3, in0=mg3, in1=at3)
                sq3 = work.tile([128, B], F16, tag="sq3", name="sq3")
                nc.vector.tensor_mul(out=sq3, in0=az3, in1=az3)
                v3 = work.tile([128, B], F16, tag="v3", name="v3")
                nc.vector.tensor_mul(out=v3, in0=h3, in1=sq3)
                nc.tensor.matmul(papp3, negIs, v3, start=False, stop=True)
                mhpp3 = work.tile([128, B], F16, tag="mhpp3", name="mhpp3")
                nc.vector.tensor_mul(out=mhpp3, in0=mg3, in1=papp3)

                # ---- layer 4: accumulate all 8 output rows ----
                p4 = psPP.tile([8, B], F32, tag="pp", name="p4")
                nc.tensor.matmul(p4, W4hs, h3, start=True, stop=False)
                nc.tensor.matmul(p4, W4zs, mtz3, start=False, stop=False)
                nc.tensor.matmul(p4, W4ts, mtt3, start=False, stop=False)
                nc.tensor.matmul(p4, W4ps, mhpp3, start=False, stop=True)
                sb4 = work.tile([8, B], F32, tag="sb4", name="sb4")
                nc.scalar.copy(out=sb4, in_=p4)
                ofull = out_d[:]
                o8 = bass.AP(
                    tensor=ofull.tensor,
                    offset=ofull.offset + t0 * B,
                    ap=[[B, 2], [NSH, 4], [1, B]],
                )
                nc.sync.dma_start(out=o8, in_=sb4)

    nc.finalize()
    return nc


_NC_CACHE = None


def _get_nc():
    global _NC_CACHE
    if _NC_CACHE is None:
        _NC_CACHE = _build()
    return _NC_CACHE


def kernel(**inputs):
    global LAST_RESULT
    f = np.float32
    f16 = np.float16
    x = np.asarray(inputs["x"], dtype=f)
    W1 = np.asarray(inputs["W1"], dtype=f)
    b1 = np.asarray(inputs["b1"], dtype=f)
    W2 = np.asarray(inputs["W2"], dtype=f)
    b2 = np.asarray(inputs["b2"], dtype=f)
    W3 = np.asarray(inputs["W3"], dtype=f)
    b3 = np.asarray(inputs["b3"], dtype=f)
    W4 = np.asarray(inputs["W4"], dtype=f)
    b4 = np.asarray(inputs["b4"], dtype=f)

    xT = np.ascontiguousarray(x.T)  # [3, N]
    w4 = W4[:, 0].astype(f)
    SQ2 = np.sqrt(2.0).astype(f)

    W4h = np.zeros((128, 8), f)
    W4h[0:64, 0] = w4
    W4h[64:128, 4] = w4
    W4z = np.zeros((128, 8), f)
    W4z[0:64, 1] = -w4 / SQ2
    W4z[64:128, 5] = -w4 / SQ2
    W4t = np.zeros((128, 8), f)
    W4t[0:64, 2] = -w4
    W4t[64:128, 6] = -w4
    W4p = np.zeros((128, 8), f)
    W4p[0:64, 3] = -w4
    W4p[64:128, 7] = -w4

    common = {
        "W1": W1,
        "W2": W2.astype(f16),
        "W2z": (SQ2 * W1[0][:, None] * W2).astype(f16),
        "W2t": (W1[1][:, None] * W2).astype(f16),
        "W2zz": (-2.0 * (W1[0] ** 2)[:, None] * W2).astype(f16),
        "W3": W3.astype(f16),
        "W3n": (-W3).astype(f16),
        "W4h": W4h.astype(f16),
        "W4z": W4z.astype(f16),
        "W4t": W4t.astype(f16),
        "W4p": W4p.astype(f16),
        "negI": (-np.eye(128)).astype(f16),
        "b1": np.ascontiguousarray(b1.reshape(128, 1)),
        "b2": np.ascontiguousarray(b2.reshape(128, 1)),
        "b3d": np.ascontiguousarray(np.concatenate([b3, b3]).reshape(128, 1)),
    }
    in_maps = [
        dict(common, xT=np.ascontiguousarray(xT[:, i * NSH : (i + 1) * NSH]))
        for i in range(NCORES)
    ]

    nc = _get_nc()
    res = run_bass_kernel_spmd(nc, in_maps, list(range(NCORES)), trace=TRACE)
    LAST_RESULT = res

    full = np.concatenate(
        [res.results[i]["out"] for i in range(NCORES)], axis=1
    )  # [4, N] rows (T, Tz, Tt, Tpp)
    out = np.ascontiguousarray(full.T).astype(f)
    out[:, 0] += b4[0]
    return out


# revision 6
# speedup vs baseline: 1.8692x; 1.0068x over previous
"""Trainium2 Bass kernel v2: PINN MLP forward + JVP derivs (T, Tz, Tt, Tzz).

Math per point (feature-major), layer l: h = tanh(a), g = 1 - h^2:
  forward:      a_{l+1} = W^T h_l
  1st deriv:    a'_{l+1} = W^T (g_l * a'_l)         (z and t directions)
  2nd deriv(z): a''_{l+1} = W^T (g_l * (a''_l - 2 h_l a'_l^2))
L1 derivative seeds fold into host-precomputed W2z/W2t/W2zz.

Perf structure vs v1:
  - All L2+ matmuls in fp16 (1 cyc/row on PE vs 4 for fp32); L1 in fp32r.
  - z-chain carries sqrt(2) (folded into W2z and the W4 output column) so
    the 2*az^2 term is a plain square -- no scale op.
  - g is never materialized: ops use (h^2 - 1) via scalar_tensor_tensor,
    with the sign folded into -W3 / -W4 weight copies.
  - (az|at) PSUM pairs are converted to fp16 SBUF by one ACT copy, making
    downstream DVE ops 2-byte (2x mode) and GPSIMD-eligible (no PSUM port).
  - i = app - v runs on the PE as an accumulating (-I) matmul.
  - Layer 3 (width 64) processes TILE PAIRS packed on partitions; layer 4
    is 4 accumulating [128,8] matmuls producing all 8 output rows.
  - Elementwise is load-balanced: ACT (tanh+converts), DVE (squares,
    products), GPSIMD (stt forms), PE (adds).

Sharding: pure data parallel, 262144 points -> 8 cores x 32768.
"""

import sys

import numpy as np

sys.path.insert(0, "/opt/trn_rl_repo")

import concourse.bass as bass
import concourse.bacc as bacc
import concourse.tile as tile
from concourse import mybir
from concourse.bass_utils import run_bass_kernel_spmd

N = 262144
NCORES = 8
NSH = N // NCORES  # 32768 points per core
B = 512  # points per tile
NT = NSH // B  # 64 tiles
NP = NT // 2  # 32 tile pairs
CH = 4096  # x chunk (points) per input DMA
TPC = CH // B  # tiles per chunk

F32 = mybir.dt.float32
F32R = mybir.dt.float32r
F16 = mybir.dt.float16

TRACE = False
LAST_RESULT = None


def _r(ap):
    return ap.bitcast(F32R)


def _build():
    nc = bacc.Bacc(None, target_bir_lowering=False)

    xT = nc.declare_dram_parameter("xT", [3, NSH], F32R, isOutput=False)
    W1 = nc.declare_dram_parameter("W1", [3, 128], F32R, isOutput=False)
    W2 = nc.declare_dram_parameter("W2", [128, 128], F16, isOutput=False)
    W2z = nc.declare_dram_parameter("W2z", [128, 128], F16, isOutput=False)
    W2t = nc.declare_dram_parameter("W2t", [128, 128], F16, isOutput=False)
    W2zz = nc.declare_dram_parameter("W2zz", [128, 128], F16, isOutput=False)
    W3 = nc.declare_dram_parameter("W3", [128, 64], F16, isOutput=False)
    W3n = nc.declare_dram_parameter("W3n", [128, 64], F16, isOutput=False)
    W4h = nc.declare_dram_parameter("W4h", [128, 8], F16, isOutput=False)
    W4z = nc.declare_dram_parameter("W4z", [128, 8], F16, isOutput=False)
    W4t = nc.declare_dram_parameter("W4t", [128, 8], F16, isOutput=False)
    W4p = nc.declare_dram_parameter("W4p", [128, 8], F16, isOutput=False)
    negI = nc.declare_dram_parameter("negI", [128, 128], F16, isOutput=False)
    b1 = nc.declare_dram_parameter("b1", [128, 1], F32, isOutput=False)
    b2 = nc.declare_dram_parameter("b2", [128, 1], F32, isOutput=False)
    b3d = nc.declare_dram_parameter("b3d", [128, 1], F32, isOutput=False)
    out_d = nc.declare_dram_parameter("out", [4, NSH], F32, isOutput=True)

    Tanh = mybir.ActivationFunctionType.Tanh
    mult = mybir.AluOpType.mult
    sub = mybir.AluOpType.subtract
    add = mybir.AluOpType.add

    with tile.TileContext(nc) as tc:
        with (
            tc.tile_pool(name="consts", bufs=1) as consts,
            tc.tile_pool(name="xin", bufs=2) as xin,
            tc.tile_pool(name="work", bufs=4) as work,
            tc.tile_pool(name="psA", bufs=2, space="PSUM") as psA,
            tc.tile_pool(name="psZT", bufs=2, space="PSUM") as psZT,
            tc.tile_pool(name="psPP", bufs=2, space="PSUM") as psPP,
        ):
            W1s = consts.tile([3, 128], F32R)
            nc.sync.dma_start(out=W1s, in_=W1[:])
            W2s = consts.tile([128, 128], F16)
            nc.sync.dma_start(out=W2s, in_=W2[:])
            W2zs = consts.tile([128, 128], F16)
            nc.sync.dma_start(out=W2zs, in_=W2z[:])
            W2ts = consts.tile([128, 128], F16)
            nc.sync.dma_start(out=W2ts, in_=W2t[:])
            W2zzs = consts.tile([128, 128], F16)
            nc.sync.dma_start(out=W2zzs, in_=W2zz[:])
            W3s = consts.tile([128, 64], F16)
            nc.sync.dma_start(out=W3s, in_=W3[:])
            W3ns = consts.tile([128, 64], F16)
            nc.sync.dma_start(out=W3ns, in_=W3n[:])
            W4hs = consts.tile([128, 8], F16)
            nc.sync.dma_start(out=W4hs, in_=W4h[:])
            W4zs = consts.tile([128, 8], F16)
            nc.sync.dma_start(out=W4zs, in_=W4z[:])
            W4ts = consts.tile([128, 8], F16)
            nc.sync.dma_start(out=W4ts, in_=W4t[:])
            W4ps = consts.tile([128, 8], F16)
            nc.sync.dma_start(out=W4ps, in_=W4p[:])
            negIs = consts.tile([128, 128], F16)
            nc.sync.dma_start(out=negIs, in_=negI[:])
            b1s = consts.tile([128, 1], F32)
            nc.sync.dma_start(out=b1s, in_=b1[:])
            b2s = consts.tile([128, 1], F32)
            nc.sync.dma_start(out=b2s, in_=b2[:])
            b3ds = consts.tile([128, 1], F32)
            nc.sync.dma_start(out=b3ds, in_=b3d[:])

            xc = None

            def half_tile(t):
                """L1+L2 for one tile; returns (h2, mtz2, mtt2, mhpp2)."""
                nonlocal xc
                ci = t % TPC
                if ci == 0:
                    c0 = (t // TPC) * CH
                    xc = xin.tile([3, CH], F32R, tag="xc", name="xc")
                    nc.sync.dma_start(out=xc, in_=xT[:, c0 : c0 + CH])
                xs = xc[:, ci * B : (ci + 1) * B]

                # ---- layer 1 ----
                pa1 = psA.tile([128, B], F32, tag="pa", name="pa1")
                nc.tensor.matmul(pa1, W1s, xs)
                h1 = work.tile([128, B], F16, tag="h1", name="h1")
                nc.scalar.activation(out=h1, in_=pa1, func=Tanh, bias=b1s)
                hh1 = work.tile([128, B], F16, tag="hh1", name="hh1")
                nc.vector.tensor_mul(out=hh1, in0=h1, in1=h1)
                g1 = work.tile([128, B], F16, tag="g1", name="g1")
                nc.vector.tensor_scalar(
                    out=g1, in0=hh1, scalar1=-1.0, scalar2=1.0, op0=mult, op1=add
                )
                u1 = work.tile([128, B], F16, tag="u1", name="u1")
                nc.vector.tensor_mul(out=u1, in0=h1, in1=g1)

                # ---- layer 2 matmuls ----
                pa2 = psA.tile([128, B], F32, tag="pa", name="pa2")
                nc.tensor.matmul(pa2, W2s, h1)
                dzt2 = psZT.tile([128, 2 * B], F32, tag="zt", name="dzt2")
                nc.tensor.matmul(dzt2[:, 0:B], W2zs, g1)  # az' = sqrt2*az
                nc.tensor.matmul(dzt2[:, B : 2 * B], W2ts, g1)  # at
                papp2 = psPP.tile([128, B], F32, tag="pp", name="papp2")
                nc.tensor.matmul(papp2, W2zzs, u1, start=True, stop=False)

                # ---- layer 2 pointwise ----
                h2 = work.tile([128, B], F16, tag="h2", name="h2")
                nc.scalar.activation(out=h2, in_=pa2, func=Tanh, bias=b2s)
                zt2 = work.tile([128, 2 * B], F16, tag="zt2", name="zt2")
                nc.scalar.copy(out=zt2, in_=dzt2)  # az'|at -> fp16 SBUF
                az2 = zt2[:, 0:B]
                at2 = zt2[:, B : 2 * B]
                mg2 = work.tile([128, B], F16, tag="mg2", name="mg2")
                nc.vector.tensor_mul(out=mg2, in0=h2, in1=h2)
                # mg2 = h2^2 - 1 = -g2
                nc.vector.tensor_scalar(
                    out=mg2, in0=mg2, scalar1=1.0, scalar2=None, op0=sub
                )
                # -t2z' = mg2 * az', -t2t = mg2 * at  (GPSIMD)
                mtz2 = work.tile([128, B], F16, tag="mtz2", name="mtz2")
                nc.gpsimd.tensor_mul(out=mtz2, in0=mg2, in1=az2)
                mtt2 = work.tile([128, B], F16, tag="mtt2", name="mtt2")
                nc.gpsimd.tensor_mul(out=mtt2, in0=mg2, in1=at2)
                sq2 = work.tile([128, B], F16, tag="sq2", name="sq2")
                nc.vector.tensor_mul(out=sq2, in0=az2, in1=az2)  # 2*az^2
                v2 = work.tile([128, B], F16, tag="v2", name="v2")
                nc.vector.tensor_mul(out=v2, in0=h2, in1=sq2)
                # papp2 <- a2'' - v2  (PE accumulate)
                nc.tensor.matmul(papp2, negIs, v2, start=False, stop=True)
                return h2, mtz2, mtt2, mg2, papp2

            for p in range(NP):
                t0 = 2 * p
                ha, tza, tta, mga, p2a = half_tile(t0)
                hb, tzb, ttb, mgb, p2b = half_tile(t0 + 1)
                # -h2'' = mg2 * i2; deferred past the next half tile so the
                # DVE never head-of-line blocks on the PE's (-I) accumulate.
                ppa = work.tile([128, B], F16, tag="mhpp2", name="mhpp2a")
                nc.vector.tensor_mul(out=ppa, in0=mga, in1=p2a)
                ppb = work.tile([128, B], F16, tag="mhpp2", name="mhpp2b")
                nc.vector.tensor_mul(out=ppb, in0=mgb, in1=p2b)

                # ---- layer 3 matmuls (pair-packed on partitions) ----
                pa3 = psA.tile([128, B], F32, tag="pa", name="pa3")
                nc.tensor.matmul(pa3[0:64], W3s, ha)
                nc.tensor.matmul(pa3[64:128], W3s, hb)
                dzt3 = psZT.tile([128, 2 * B], F32, tag="zt", name="dzt3")
                nc.tensor.matmul(dzt3[0:64, 0:B], W3ns, tza)  # a3z' tile a
                nc.tensor.matmul(dzt3[64:128, 0:B], W3ns, tzb)
                nc.tensor.matmul(dzt3[0:64, B : 2 * B], W3ns, tta)  # a3t
                nc.tensor.matmul(dzt3[64:128, B : 2 * B], W3ns, ttb)
                papp3 = psPP.tile([128, B], F32, tag="pp", name="papp3")
                nc.tensor.matmul(papp3[0:64], W3ns, ppa, start=True, stop=False)
                nc.tensor.matmul(papp3[64:128], W3ns, ppb, start=True, stop=False)

                # ---- layer 3 pointwise ----
                h3 = work.tile([128, B], F16, tag="h3", name="h3")
                nc.scalar.activation(out=h3, in_=pa3, func=Tanh, bias=b3ds)
                zt3 = work.tile([128, 2 * B], F16, tag="zt3", name="zt3")
                nc.scalar.copy(out=zt3, in_=dzt3)
                az3 = zt3[:, 0:B]
                at3 = zt3[:, B : 2 * B]
                mg3 = work.tile([128, B], F16, tag="mg3", name="mg3")
                nc.vector.tensor_mul(out=mg3, in0=h3, in1=h3)
                nc.vector.tensor_scalar(
                    out=mg3, in0=mg3, scalar1=1.0, scalar2=None, op0=sub
                )
                mtz3 = work.tile([128, B], F16, tag="mtz3", name="mtz3")
                nc.gpsimd.tensor_mul(out=mtz3, in0=mg3, in1=az3)
                mtt3 = work.tile([128, B], F16, tag="mtt3", name="mtt3")
                nc.gpsimd.tensor_mul(out=mtt3, in0=mg3, in1=at3)
                sq3 = work.tile([128, B], F16, tag="sq3", name="sq3")
                nc.vector.tensor_mul(out=sq3, in0=az3, in1=az3)
                v3 = work.tile([128, B], F16, tag="v3", name="v3")
                nc.vector.tensor_mul(out=v3, in0=h3, in1=sq3)
                nc.tensor.matmul(papp3, negIs, v3, start=False, stop=True)
                mhpp3 = work.tile([128, B], F16, tag="mhpp3", name="mhpp3")
                nc.vector.tensor_mul(out=mhpp3, in0=mg3, in1=papp3)

                # ---- layer 4: accumulate all 8 output rows ----
                p4 = psPP.tile([8, B], F32, tag="pp", name="p4")
                nc.tensor.matmul(p4, W4hs, h3, start=True, stop=False)
                nc.tensor.matmul(p4, W4zs, mtz3, start=False, stop=False)
                nc.tensor.matmul(p4, W4ts, mtt3, start=False, stop=False)
                nc.tensor.matmul(p4, W4ps, mhpp3, start=False, stop=True)
                sb4 = work.tile([8, B], F32, tag="sb4", name="sb4")
                nc.scalar.copy(out=sb4, in_=p4)
                ofull = out_d[:]
                o8 = bass.AP(
                    tensor=ofull.tensor,
                    offset=ofull.offset + t0 * B,
                    ap=[[B, 2], [NSH, 4], [1, B]],
                )
                nc.sync.dma_start(out=o8, in_=sb4)

    nc.finalize()
    return nc


_NC_CACHE = None


def _get_nc():
    global _NC_CACHE
    if _NC_CACHE is None:
        _NC_CACHE = _build()
    return _NC_CACHE


def kernel(**inputs):
    global LAST_RESULT
    f = np.float32
    f16 = np.float16
    x = np.asarray(inputs["x"], dtype=f)
    W1 = np.asarray(inputs["W1"], dtype=f)
    b1 = np.asarray(inputs["b1"], dtype=f)
    W2 = np.asarray(inputs["W2"], dtype=f)
    b2 = np.asarray(inputs["b2"], dtype=f)
    W3 = np.asarray(inputs["W3"], dtype=f)
    b3 = np.asarray(inputs["b3"], dtype=f)
    W4 = np.asarray(inputs["W4"], dtype=f)
    b4 = np.asarray(inputs["b4"], dtype=f)

    xT = np.ascontiguousarray(x.T)  # [3, N]
    w4 = W4[:, 0].astype(f)
    SQ2 = np.sqrt(2.0).astype(f)

    W4h = np.zeros((128, 8), f)
    W4h[0:64, 0] = w4
    W4h[64:128, 4] = w4
    W4z = np.zeros((128, 8), f)
    W4z[0:64, 1] = -w4 / SQ2
    W4z[64:128, 5] = -w4 / SQ2
    W4t = np.zeros((128, 8), f)
    W4t[0:64, 2] = -w4
    W4t[64:128, 6] = -w4
    W4p = np.zeros((128, 8), f)
    W4p[0:64, 3] = -w4
    W4p[64:128, 7] = -w4

    common = {
        "W1": W1,
        "W2": W2.astype(f16),
        "W2z": (SQ2 * W1[0][:, None] * W2).astype(f16),
        "W2t": (W1[1][:, None] * W2).astype(f16),
        "W2zz": (-2.0 * (W1[0] ** 2)[:, None] * W2).astype(f16),
        "W3": W3.astype(f16),
        "W3n": (-W3).astype(f16),
        "W4h": W4h.astype(f16),
        "W4z": W4z.astype(f16),
        "W4t": W4t.astype(f16),
        "W4p": W4p.astype(f16),
        "negI": (-np.eye(128)).astype(f16),
        "b1": np.ascontiguousarray(b1.reshape(128, 1)),
        "b2": np.ascontiguousarray(b2.reshape(128, 1)),
        "b3d": np.ascontiguousarray(np.concatenate([b3, b3]).reshape(128, 1)),
    }
    in_maps = [
        dict(common, xT=np.ascontiguousarray(xT[:, i * NSH : (i + 1) * NSH]))
        for i in range(NCORES)
    ]

    nc = _get_nc()
    res = run_bass_kernel_spmd(nc, in_maps, list(range(NCORES)), trace=TRACE)
    LAST_RESULT = res

    full = np.concatenate(
        [res.results[i]["out"] for i in range(NCORES)], axis=1
    )  # [4, N] rows (T, Tz, Tt, Tpp)
    out = np.ascontiguousarray(full.T).astype(f)
    out[:, 0] += b4[0]
    return out
